# revision 62
# baseline (speedup 1.0000x reference)
"""Trainium2 Bass kernel for nn_Encoder_70781061038947.

Math: row b's output depends on x[b, :] only through its 16 sign bits
(root k has radius R if x[b,k] > 0 else 1/R, phase shuffle_vector[k]).
Evaluate the monic degree-16 polynomial at the 17th roots of unity in
LOG space: log P(t_m) = const_m + sum_k s_k c_mk with s_k = +-1 signs,
c_mk = (Log(t_m - R e^{i th_k}) - Log(t_m - e^{i th_k}/R)) / 2.  One
bf16 hi/lo matmul pair per bank of 1024 rows gives log-magnitudes and
phases; P = K_m exp(lr) (cos li, sin li) with K_m = exp(const_m) folded
into the inverse-DFT matrix.

The host pre-extracts sign bits ((x & 0x8000) | 0x3F80 on the bf16
image: exact +-1.0) and lays them out transposed, so the device input
DMA is a plain contiguous load.  Work is issued in 2048-row
superchunks to halve per-instruction dispatch overheads:
  SP  : input load, bf16 output store
  PE  : 2 hi/lo matmul pairs, 6 value transposes, 6 inverse-DFT matmuls
  ACT : exp, sin2pi twice, square, PSUM->SBUF value copy (all in the
        exp_and_friends HW table: Sin is rewritten to Sin2pi in BIR),
        plus per-4-superchunk batches of norm sqrts (the only
        activation-table swaps)
  DVE : phase range reduction (magic round, subtract, bitwise abs),
        norm reduce + reciprocal, fused normalize+PSUM-drain (bf16 out)
  Pool: |K|-weighted eval magnitude, two eval-value products

Sharding: pure data parallel over B across 8 cores (32768 rows each);
the small tables derived from shuffle_vector are replicated inputs.
"""

import types

import numpy as np
import ml_dtypes

import concourse.bacc as bacc
import concourse.bass as bass
import concourse.mybir as mybir
import concourse.bass_utils as bass_utils
import concourse.hw_specs as hw_specs
import concourse.tile as tile

B = 262144
K = 16
M = 17                      # evaluation points (17th roots of unity)
W = 2 * M                   # 34 outputs per row
NCORES = 8
RPC = B // NCORES           # 32768 rows per core
P = 128
CPB = RPC // P              # 256 rows per partition
TPC = 8                     # subtiles per 1024-row bank
SC = CPB // (2 * TPC)       # 16 superchunks (2048 rows each)
GRP = 4                     # superchunks per sqrt batch
FO = TPC * W                # 272 outputs per bank of rows
SFO = 2 * FO                # 544 outputs per superchunk

_cached = None


def _tables(shuffle_vector: np.ndarray):
    sv = np.asarray(shuffle_vector, dtype=np.float64)
    R = np.sqrt(1.0 + np.sin(np.pi / K))
    t = np.exp(2j * np.pi * np.arange(M) / M)
    bf16 = ml_dtypes.bfloat16

    zhi = R * np.exp(1j * sv)
    zlo = np.exp(1j * sv) / R
    a = np.log(t[None, :] - zhi[:, None])          # (K, M)
    b = np.log(t[None, :] - zlo[:, None])
    c = (a - b) / 2                                # +-1 sign weights
    Km = np.exp(((a + b) / 2).sum(axis=0))         # (M,)

    # C table: cols 0..16 -> Re c; cols 17..33 -> Im c / 2pi (sin2pi units)
    Cmat = np.concatenate([c.real, c.imag / (2 * np.pi)], axis=1)  # (16, 34)

    # block-diagonal 8x copy: row 16u+k, col 34u+q
    Cbig = np.zeros((P, FO), np.float64)
    for u in range(TPC):
        Cbig[u * K:(u + 1) * K, u * W:(u + 1) * W] = Cmat
    chi = Cbig.astype(bf16)
    clo = (Cbig - chi.astype(np.float64)).astype(bf16)

    # inverse DFT with K_m folded: c_d = sum_m Q_m * (K_m w_md),
    # w_md = exp(-2pi i (K-d) m / 17) / 17; rows [Qre(17); Qim(17)],
    # cols re/im interleaved.
    w = np.exp(-2j * np.pi * ((K - np.arange(M)[None, :])
                              * np.arange(M)[:, None]) / M) / M
    WKc = Km[:, None] * w
    W2K = np.zeros((W, W), np.float64)
    W2K[:M, 0::2] = WKc.real
    W2K[:M, 1::2] = WKc.imag
    W2K[M:, 0::2] = -WKc.imag
    W2K[M:, 1::2] = WKc.real

    w2k3 = np.zeros((3 * W, 3 * W), np.float64)
    for j in range(3):
        w2k3[j * W:(j + 1) * W, j * W:(j + 1) * W] = W2K
    w2k2 = np.zeros((2 * W, 2 * W), np.float64)
    for j in range(2):
        w2k2[j * W:(j + 1) * W, j * W:(j + 1) * W] = W2K

    rkm = np.tile(np.abs(Km)[None, :], (P, 1)).astype(bf16)  # (128, 17)
    quarter = np.full((P, 1), 0.25, np.float32)
    ident_bf = np.eye(P, dtype=bf16)

    return {
        "chi": chi,
        "clo": clo,
        "w2k3": w2k3.astype(bf16),
        "w2k2": w2k2.astype(bf16),
        "rkm": rkm,
        "quarter": quarter,
        "identb": ident_bf,
    }


def _prep_signs(x_bf: np.ndarray) -> np.ndarray:
    """Host-side: sign bits to exact +-1.0 bf16, transposed per superchunk.

    Output [P, SC*2*P] with element (q=16u+k, (sc, cb, p)) = sign of
    x[row, k] for row = p*CPB + (2*sc + cb)*TPC + u.
    """
    bf16 = ml_dtypes.bfloat16
    s16 = (x_bf.view(np.uint16) & np.uint16(0x8000)) | np.uint16(0x3F80)
    s = s16.view(bf16).reshape(P, SC, 2, TPC, K)          # p, sc, cb, u, k
    s = np.ascontiguousarray(s.transpose(3, 4, 1, 2, 0))  # u, k, sc, cb, p
    return s.reshape(P, SC * 2 * P)


def _poison_act_tables(arch: str):
    """Confine Exp/Sin/Square/Copy to the exp_and_friends HW table so the
    main loop runs without activation-table reloads (Sin is rewritten to
    Sin2pi, which lives in that table).  Sqrt keeps sqrt_and_friends."""
    AF = mybir.ActivationFunctionType
    tabs = hw_specs.get_activation_tables(arch)
    for name, s in tabs.items():
        if name not in ("exp_and_friends", "sqrt_and_friends"):
            s.discard(AF.Sin)
            s.discard(AF.Exp)
            s.discard(AF.Square)
            s.discard(AF.Copy)
            s.discard(AF.Identity)
            s.discard(AF.Sqrt)
        elif name == "exp_and_friends":
            s.discard(AF.Sqrt)
        else:
            s.discard(AF.Exp)
    tabs["exp_and_friends"].add(AF.Sin)


def _build_module(rpc=RPC):
    f32 = mybir.dt.float32
    i32 = mybir.dt.int32
    bf = mybir.dt.bfloat16
    AF = mybir.ActivationFunctionType
    OP = mybir.AluOpType
    MAGIC = float(1.5 * 2 ** 23)

    nc = bacc.Bacc("TRN2", target_bir_lowering=False, debug=False)
    _poison_act_tables(nc.m.arch)

    sgnt_d = nc.dram_tensor("sgnt", [P, SC * 2 * P], bf, kind="ExternalInput")
    chi_d = nc.dram_tensor("chi", [P, FO], bf, kind="ExternalInput")
    clo_d = nc.dram_tensor("clo", [P, FO], bf, kind="ExternalInput")
    w2k3_d = nc.dram_tensor("w2k3", [3 * W, 3 * W], bf, kind="ExternalInput")
    w2k2_d = nc.dram_tensor("w2k2", [2 * W, 2 * W], bf, kind="ExternalInput")
    rkm_d = nc.dram_tensor("rkm", [P, M], bf, kind="ExternalInput")
    quarter_d = nc.dram_tensor("quarter", [P, 1], f32, kind="ExternalInput")
    identb_d = nc.dram_tensor("identb", [P, P], bf, kind="ExternalInput")
    out_d = nc.dram_tensor("out", [rpc, W], bf, kind="ExternalOutput")

    sgnt_v = sgnt_d.ap()
    # row (p*CPB + c) -> partition p, column c
    out_v = out_d.ap().rearrange("(p c) e -> p (c e)", p=P)  # [128, CPB*34]

    with tile.TileContext(nc) as tc:
        with (
            tc.tile_pool(name="const", bufs=1) as cp,
            tc.tile_pool(name="sb", bufs=4) as sp,
            tc.tile_pool(name="hold", bufs=2) as hp,
            tc.tile_pool(name="ps", bufs=1, space="PSUM") as pp,
        ):
            chi_sb = cp.tile([P, FO], bf)
            nc.scalar.dma_start(out=chi_sb[:], in_=chi_d.ap())
            clo_sb = cp.tile([P, FO], bf)
            nc.scalar.dma_start(out=clo_sb[:], in_=clo_d.ap())
            w2k3_sb = cp.tile([3 * W, 3 * W], bf)
            nc.scalar.dma_start(out=w2k3_sb[:], in_=w2k3_d.ap())
            w2k2_sb = cp.tile([2 * W, 2 * W], bf)
            nc.scalar.dma_start(out=w2k2_sb[:], in_=w2k2_d.ap())
            rkm_sb = cp.tile([P, M], bf)
            nc.scalar.dma_start(out=rkm_sb[:], in_=rkm_d.ap())
            quarter = cp.tile([P, 1], f32)
            nc.scalar.dma_start(out=quarter[:], in_=quarter_d.ap())
            identb = cp.tile([P, P], bf)
            nc.scalar.dma_start(out=identb[:], in_=identb_d.ap())

            def phase_a(sc, erwg, gidx):
                """Load + matmuls + transcendentals + weighted magnitudes."""
                sgn = sp.tile([P, 2 * P], bf, tag="sgn")
                nc.sync.dma_start(
                    out=sgn[:], in_=sgnt_v[:, sc * 2 * P:(sc + 1) * 2 * P])

                # two banks of log-evals; [128, 1024] f32 = 2 PSUM banks,
                # each matmul pair confined to one bank (cols 512*cb)
                L = pp.tile([P, 1024], f32, tag="L", bufs=2)
                for cb in range(2):
                    nc.tensor.matmul(
                        out=L[:, cb * 512:cb * 512 + FO],
                        lhsT=sgn[:, cb * P:(cb + 1) * P], rhs=chi_sb[:],
                        start=True, stop=False)
                    nc.tensor.matmul(
                        out=L[:, cb * 512:cb * 512 + FO],
                        lhsT=sgn[:, cb * P:(cb + 1) * P], rhs=clo_sb[:],
                        start=False, stop=True)
                Lv = L[:].rearrange("p (c h) -> p c h", c=2)
                Lq = Lv[:, :, 0:FO].rearrange("p c (u q) -> p c u q", q=W)

                # er = |P/K| = exp(lr)
                er = hp.tile([P, SFO // 2], bf, tag="er", bufs=SC + 2)
                nc.scalar.activation(
                    out=er[:].rearrange("p (c u m) -> p c u m", c=2, m=M),
                    in_=Lq[:, :, :, 0:M], func=AF.Exp)

                # range reduction: w = y - round(y) in [-.5,.5]; a = |w|
                ph = Lq[:, :, :, M:W]
                r1 = sp.tile([P, SFO // 2], f32, tag="r1")
                nc.vector.tensor_scalar(
                    out=r1[:].rearrange("p (c u m) -> p c u m", c=2, m=M),
                    in0=ph,
                    scalar1=MAGIC, scalar2=MAGIC, op0=OP.add, op1=OP.subtract)
                w = sp.tile([P, SFO // 2], f32, tag="w")
                nc.vector.tensor_tensor(
                    out=w[:].rearrange("p (c u m) -> p c u m", c=2, m=M),
                    in0=ph,
                    in1=r1[:].rearrange("p (c u m) -> p c u m", c=2, m=M),
                    op=OP.subtract)
                aw = sp.tile([P, SFO // 2], f32, tag="aw")
                nc.vector.tensor_scalar(
                    out=aw[:].bitcast(i32), in0=w[:].bitcast(i32),
                    scalar1=0x7FFFFFFF, scalar2=None, op0=OP.bitwise_and)

                # cos/sin interleaved per subtile: cs[p, v, 0, m] = cos,
                # cs[p, v, 1, m] = sin -- sin2pi(w) and sin2pi(0.25 - |w|)
                cs = hp.tile([P, SFO], bf, tag="cs", bufs=SC + 2)
                csq = cs[:].rearrange("p (v q) -> p v q", q=W)
                nc.scalar.activation(out=csq[:, :, M:W],
                                     in_=w[:].rearrange("p (v m) -> p v m", m=M),
                                     func=AF.Sin)
                nc.scalar.activation(out=csq[:, :, 0:M],
                                     in_=aw[:].rearrange("p (v m) -> p v m", m=M),
                                     func=AF.Sin, bias=quarter[:], scale=-1.0)

                # norm via Parseval: S = sum_m (|K_m| er)^2; the weighted
                # magnitudes accumulate into a per-group tile for batched
                # square/reduce/reciprocal/sqrt
                nc.gpsimd.tensor_tensor(
                    out=erwg[:, gidx * (SFO // 2):(gidx + 1) * (SFO // 2)]
                        .rearrange("p (v m) -> p v m", m=M),
                    in0=er[:].rearrange("p (v m) -> p v m", m=M),
                    in1=rkm_sb[:].unsqueeze(1).to_broadcast([P, 2 * TPC, M]),
                    op=OP.mult)
                return er, cs

            def phase_b1(sc, er_ap, cs_ap):
                """Value products + transposes + PSUM drain (no norm dep)."""
                # Q values, bf16, packed [Qre(17) | Qim(17)] per subtile:
                # one product op, er broadcast over the re/im pair
                vc = sp.tile([P, SFO], bf, tag="vc")
                nc.gpsimd.tensor_tensor(
                    out=vc[:].rearrange("p (v r m) -> p v r m", r=2, m=M),
                    in0=er_ap.rearrange("p (v m) -> p v m", m=M)
                        .unsqueeze(2).to_broadcast([P, 2 * TPC, 2, M]),
                    in1=cs_ap.rearrange("p (v r m) -> p v r m", r=2, m=M),
                    op=OP.mult)

                # transpose values in subtile-groups of (3,3,2) per bank
                vcT = pp.tile([3 * W, 6 * P], bf, tag="vcT", bufs=2)
                widths = [3 * W, 3 * W, 2 * W] * 2
                offs = [0, 3 * W, 6 * W, FO, FO + 3 * W, FO + 6 * W]
                for j, (wdt, off) in enumerate(zip(widths, offs)):
                    nc.tensor.transpose(
                        out=vcT[0:wdt, j * P:(j + 1) * P],
                        in_=vc[:, off:off + wdt],
                        identity=identb[:])
                vcT_sb = sp.tile([3 * W, 6 * P], bf, tag="vcTs")
                nc.scalar.activation(out=vcT_sb[:], in_=vcT[:], func=AF.Copy)
                return vcT_sb

            def phase_b2(sc, vcT_sb, fac):
                """Inverse DFT + normalize + store."""
                # block-diagonal inverse-DFT; [128, 1024] f32 = 2 banks
                o_ps = pp.tile([P, 1024], f32, tag="o", bufs=1)
                for cb in range(2):
                    base = cb * 512
                    j0 = cb * 3
                    nc.tensor.matmul(
                        out=o_ps[:, base:base + 3 * W],
                        lhsT=vcT_sb[0:3 * W, j0 * P:(j0 + 1) * P],
                        rhs=w2k3_sb[:], start=True, stop=True)
                    nc.tensor.matmul(
                        out=o_ps[:, base + 3 * W:base + 6 * W],
                        lhsT=vcT_sb[0:3 * W, (j0 + 1) * P:(j0 + 2) * P],
                        rhs=w2k3_sb[:], start=True, stop=True)
                    nc.tensor.matmul(
                        out=o_ps[:, base + 6 * W:base + 8 * W],
                        lhsT=vcT_sb[0:2 * W, (j0 + 2) * P:(j0 + 3) * P],
                        rhs=w2k2_sb[:], start=True, stop=True)

                # normalize while draining PSUM: out = coeffs * fac (bf16)
                out_sb = sp.tile([P, SFO], bf, tag="osb")
                nc.vector.tensor_tensor(
                    out=out_sb[:].rearrange("p (c u e) -> p c u e", c=2, e=W),
                    in0=o_ps[:].rearrange("p (c h) -> p c h", c=2)[:, :, 0:FO]
                        .rearrange("p c (u e) -> p c u e", e=W),
                    in1=fac.rearrange("p (c u) -> p c u", c=2)
                        .unsqueeze(3).to_broadcast([P, 2, TPC, W]),
                    op=OP.mult)
                nc.sync.dma_start(
                    out=out_v[:, sc * SFO:(sc + 1) * SFO], in_=out_sb[:])

            LAG = 3
            erwgs = {}
            vts = {}
            held = {}
            for sc in range(SC):
                g = sc // GRP
                if g not in erwgs:
                    erwg_t = hp.tile([P, GRP * (SFO // 2)], f32, tag="erwg",
                                     name=f"erwg{g}")
                    erwgs[g] = erwg_t
                held[sc] = phase_a(sc, erwgs[g], sc % GRP)
            for g0 in range(0, SC, GRP):
                g1 = min(g0 + GRP, SC)
                ng = g1 - g0
                erwg = erwgs[g0 // GRP]
                sqg = hp.tile([P, GRP * (SFO // 2)], f32, tag="sqg")
                nc.scalar.activation(
                    out=sqg[:, 0:ng * (SFO // 2)],
                    in_=erwg[:, 0:ng * (SFO // 2)], func=AF.Square)
                Sg = hp.tile([P, GRP * 2 * TPC], f32, tag="Sg")
                nc.vector.tensor_reduce(
                    out=Sg[:, 0:ng * 2 * TPC],
                    in_=sqg[:, 0:ng * (SFO // 2)]
                        .rearrange("p (v m) -> p v m", m=M),
                    axis=mybir.AxisListType.X, op=OP.add)
                rSg = hp.tile([P, GRP * 2 * TPC], f32, tag="rSg")
                nc.vector.reciprocal(out=rSg[:, 0:ng * 2 * TPC],
                                     in_=Sg[:, 0:ng * 2 * TPC])
                facg = hp.tile([P, GRP * 2 * TPC], f32, tag="facg")
                nc.scalar.activation(out=facg[:, 0:ng * 2 * TPC],
                                     in_=rSg[:, 0:ng * 2 * TPC], func=AF.Sqrt,
                                     bias=0.0, scale=float(M * M))
                for gidx, sc in enumerate(range(g0, g1)):
                    er, cs = held[sc]
                    fac = facg[:, gidx * 2 * TPC:(gidx + 1) * 2 * TPC]
                    vcT_sb = phase_b1(sc, er[:], cs[:])
                    phase_b2(sc, vcT_sb, fac)

    nc.compile()

    patched = nc.to_json_bytes().replace(b'"func":"Sin"', b'"func":"Sin2pi"')
    nc.to_json_bytes = types.MethodType(lambda self: patched, nc)
    return nc


def kernel(x: np.ndarray, shuffle_vector: np.ndarray) -> np.ndarray:
    global _cached
    x = np.asarray(x)
    assert x.shape == (B, K), x.shape
    x_bf = np.ascontiguousarray(x.astype(ml_dtypes.bfloat16))

    tabs = _tables(shuffle_vector)
    if _cached is None:
        _cached = _build_module()
    nc = _cached

    shards = x_bf.reshape(NCORES, RPC, K)
    in_maps = [
        {"sgnt": _prep_signs(shards[i]), **tabs}
        for i in range(NCORES)
    ]
    res = bass_utils.run_bass_kernel_spmd(nc, in_maps, core_ids=list(range(NCORES)))
    out = np.concatenate([res.results[i]["out"] for i in range(NCORES)], axis=0)
    return (out.astype(np.float32).copy().view(np.complex64)
            .reshape(B, M).astype(np.complex128))
